# revision 1
# baseline (speedup 1.0000x reference)
"""Kernel builder for nn_DecoderAutoregAdaIN on TRN2 (single core).

Algorithm (validated in proto_np.py):
  - Cross-attn mask is diagonal => cross-attn out = (memory @ Wv.T + bv) @ Wo.T + bo,
    a per-layer constant "ca_add" (precomputed on device in the preamble).
  - KV-cache incremental decode over 64 steps; output row i collected at step i.

Layouts (partition-major activations; feature f = c*128 + p, head h = 2c + (p>=64)):
  xT / residuals  [128, (4c, 2b)] fp32
  qkvb            [128, (12ch, 2b)] bf16   ch 0-3 q, 4-7 k, 8-11 v
  KT cache        [128, (L, 4kc, 2b, 64t)] bf16
  V_psum (per l)  [128, 512] fp32: partitions (b*64+t), free (c*128+d)
  scores psum     [16, 64]  slot s(b,h) = 4*(h//2) + 2*b + (h%2)
  qblock          [128, (8e, 16s)] bf16; block e = 2c+b; live cols of block e are
                  slots {4c+2b, 4c+2b+1}; flat offset = 18*(2c+b) + hpar (step-18 seq).
"""
from contextlib import ExitStack
import numpy as np
import ml_dtypes

import concourse.bass as bass
from concourse import mybir
from concourse.alu_op_type import AluOpType as ALU

F32 = mybir.dt.float32
BF16 = mybir.dt.bfloat16
AX = mybir.AxisListType.X
ACTF = mybir.ActivationFunctionType

B, T, D, M, H, L, DFF, PERIOD = 2, 64, 512, 64, 8, 3, 2048, 30
HD = D // H
EPS = 1e-5
NCK = 4          # feature chunks of 128
NF = DFF // 128  # 16


def slot_of(b, h):
    return 4 * (h // 2) + 2 * b + (h % 2)


# ---------------------------------------------------------------- host prep
def _slopes(n):
    start = 2.0 ** (-(2.0 ** -(np.log2(n) - 3)))
    return np.array([start * start ** i for i in range(n)], dtype=np.float32)


def _pe_mask():
    pos = np.arange(PERIOD)[:, None].astype(np.float32)
    div = np.exp(np.arange(0, D, 2).astype(np.float32) * (-np.log(10000.0) / D))
    pe = np.zeros((PERIOD, D), np.float32)
    pe[:, 0::2] = np.sin(pos * div)
    pe[:, 1::2] = np.cos(pos * div)
    pe_full = np.tile(pe, (T // PERIOD + 1, 1))[:T]
    ii = np.arange(T)[:, None]
    jj = np.arange(T)[None, :]
    bias = -((ii - jj) // PERIOD).astype(np.float32)
    alibi = _slopes(H)[:, None, None] * np.where(jj <= ii, bias, 0.0)
    self_mask = np.where(jj <= ii, alibi, -1e9).astype(np.float32)  # [H,T,T]
    return pe_full, self_mask


def _wtiles(w_t, n_kc):
    """w_t [K, Mo] -> [128, n_kc, Mo]; lhsT tile (kc, mc) = arr[:, kc, mc*128:(mc+1)*128]."""
    K, Mo = w_t.shape
    assert K == n_kc * 128
    return np.ascontiguousarray(w_t.reshape(n_kc, 128, Mo).transpose(1, 0, 2))


def _bf(x):
    return np.ascontiguousarray(np.asarray(x).astype(ml_dtypes.bfloat16))


def _dup_b(x):  # append duplicated b axis of size B
    return np.ascontiguousarray(np.repeat(np.asarray(x, np.float32)[..., None], B, axis=-1))


def prep_inputs(inp):
    inp = {k: np.asarray(v, np.float32) for k, v in inp.items()}
    pe_full, self_mask = _pe_mask()
    out = {}

    out["w_qkv"] = _bf(np.stack([_wtiles(inp["sa_w"][l].T, NCK) for l in range(L)], axis=1))
    out["w_out"] = _bf(np.stack([_wtiles(inp["sa_o_w"][l].T, NCK) for l in range(L)], axis=1))
    out["w_ff1"] = _bf(np.stack([_wtiles(inp["ff1_w"][l].T, NCK) for l in range(L)], axis=1))
    out["w_ff2"] = _bf(np.stack([_wtiles(inp["ff2_w"][l].T, NF) for l in range(L)], axis=1))
    out["w_cav"] = _bf(np.stack([_wtiles(inp["ca_w"][l][2 * D:].T, NCK) for l in range(L)], axis=1))
    out["w_cao"] = _bf(np.stack([_wtiles(inp["ca_o_w"][l].T, NCK) for l in range(L)], axis=1))
    out["w_adain"] = _bf(_wtiles(inp["adain_w"].T, NCK))      # [128, 4, 1024]
    out["w_mm"] = _bf(inp["mm_w"].T)                          # [64, 512]
    out["w_mmr"] = _bf(_wtiles(inp["mmr_w"].T, NCK))          # [128, 4, 64]

    def pm(v):  # [512] -> [128, 4]
        return np.ascontiguousarray(v.reshape(NCK, 128).T)

    out["b_qkv"] = _dup_b(np.stack([inp["sa_b"][l].reshape(12, 128).T for l in range(L)], axis=1))
    out["b_out"] = _dup_b(np.stack([pm(inp["sa_o_b"][l]) for l in range(L)], axis=1))
    out["b_ff1"] = _dup_b(np.stack([inp["ff1_b"][l].reshape(NF, 128).T for l in range(L)], axis=1))
    out["b_ff2"] = _dup_b(np.stack([pm(inp["ff2_b"][l]) for l in range(L)], axis=1))
    out["b_cav"] = _dup_b(np.stack([pm(inp["ca_b"][l][2 * D:]) for l in range(L)], axis=1))
    out["b_cao"] = _dup_b(np.stack([pm(inp["ca_o_b"][l]) for l in range(L)], axis=1))
    out["b_adain"] = _dup_b(inp["adain_b"].reshape(8, 128).T)
    out["b_mm"] = _dup_b(pm(inp["mm_b"]))
    out["b_mmr"] = np.ascontiguousarray(np.repeat(inp["mmr_b"][:, None], B, axis=1))
    g = np.stack([np.stack([pm(inp["ln_g"][l, j]) for j in range(3)]) for l in range(L)])  # [L,3,128,4]
    bt = np.stack([np.stack([pm(inp["ln_b"][l, j]) for j in range(3)]) for l in range(L)])
    out["g_ln"] = _dup_b(g.transpose(2, 0, 1, 3))   # [128, L, 3, 4, 2]
    out["bt_ln"] = _dup_b(bt.transpose(2, 0, 1, 3))

    out["pe_t"] = np.ascontiguousarray(pe_full.T.reshape(NCK, 128, T).transpose(1, 0, 2))  # [128,4,64]
    mask = np.zeros((16, T, T), np.float32)
    for b in range(B):
        for h in range(H):
            mask[slot_of(b, h)] = self_mask[h]
    out["maskt"] = mask
    out["ident_bf"] = _bf(np.eye(128, dtype=np.float32))
    out["ident_f32"] = np.eye(128, dtype=np.float32)
    out["ones_f32"] = np.ones((128, 128), np.float32)

    out["content_code"] = np.ascontiguousarray(inp["content_code"])
    out["style_code"] = np.ascontiguousarray(inp["style_code"])
    out["init_state"] = np.ascontiguousarray(inp["init_state"])
    return out


def input_specs():
    """name -> (shape, np dtype) for DRAM ExternalInputs."""
    bf, f32 = ml_dtypes.bfloat16, np.float32
    return {
        "w_qkv": ((128, L, NCK, 3 * D), bf), "w_out": ((128, L, NCK, D), bf),
        "w_ff1": ((128, L, NCK, DFF), bf), "w_ff2": ((128, L, NF, D), bf),
        "w_cav": ((128, L, NCK, D), bf), "w_cao": ((128, L, NCK, D), bf),
        "w_adain": ((128, NCK, 2 * D), bf), "w_mm": ((64, D), bf),
        "w_mmr": ((128, NCK, M), bf),
        "b_qkv": ((128, L, 12, B), f32), "b_out": ((128, L, NCK, B), f32),
        "b_ff1": ((128, L, NF, B), f32), "b_ff2": ((128, L, NCK, B), f32),
        "b_cav": ((128, L, NCK, B), f32), "b_cao": ((128, L, NCK, B), f32),
        "b_adain": ((128, 8, B), f32), "b_mm": ((128, NCK, B), f32),
        "b_mmr": ((64, B), f32),
        "g_ln": ((128, L, 3, NCK, B), f32), "bt_ln": ((128, L, 3, NCK, B), f32),
        "pe_t": ((128, NCK, T), f32), "maskt": ((16, T, T), f32),
        "ident_bf": ((128, 128), bf), "ident_f32": ((128, 128), f32),
        "ones_f32": ((128, 128), f32),
        "content_code": ((B, T, D), f32), "style_code": ((B, D), f32),
        "init_state": ((B, M), f32),
    }


# ---------------------------------------------------------------- builder
def build(tc, ins, outs, n_steps=T, dyn_loop=False, taps=None, tap_at=(0, 0), staggered=False):
    """Emit the program. ins/outs: dict name->AP (DRAM). taps: dict of debug
    DRAM output APs keyed by tensor name (only used when dyn_loop=False);
    tap_at = (step, layer)."""
    nc = tc.nc
    ctx = ExitStack()
    taps = taps or {}
    tap_i, tap_l = tap_at

    cp = ctx.enter_context(tc.tile_pool(name="consts", bufs=1))
    sp = ctx.enter_context(tc.tile_pool(name="state", bufs=1))
    ap_ = ctx.enter_context(tc.tile_pool(name="act", bufs=2))

    dma = nc.sync.dma_start
    TT = nc.vector.tensor_tensor
    TS = nc.vector.tensor_scalar
    CP = nc.vector.tensor_copy

    def load(pool, name):
        src = ins[name]
        t = pool.tile(list(src.shape), src.dtype, tag=name)
        dma(t[:], src[:])
        return t

    w_mm = load(cp, "w_mm")
    b_qkv = load(cp, "b_qkv"); b_out = load(cp, "b_out")
    b_ff1 = load(cp, "b_ff1"); b_ff2 = load(cp, "b_ff2")
    b_mm = load(cp, "b_mm"); b_mmr = load(cp, "b_mmr")
    g_ln = load(cp, "g_ln"); bt_ln = load(cp, "bt_ln")
    pe_t = load(cp, "pe_t"); maskt = load(cp, "maskt")
    ident_bf = load(cp, "ident_bf"); ident_f32 = load(cp, "ident_f32")
    ones_f32 = load(cp, "ones_f32")

    KT = sp.tile([128, L, NCK, B, T], BF16, tag="KT")
    V_row = sp.tile([128, L, D], BF16, tag="V_row")
    embT = sp.tile([128, NCK, B, T + 1], F32, tag="embT")
    ca_addT = sp.tile([128, L, NCK, B, T], F32, tag="ca_addT")
    out_sb = sp.tile([64, B, T], F32, tag="out_sb")
    qblock = sp.tile([128, 8 * 16], BF16, tag="qblock")
    vcol = sp.tile([128, NCK, B, T], BF16, tag="vcol")

    nc.vector.memset(KT[:], 0.0)
    nc.vector.memset(out_sb[:], 0.0)
    nc.vector.memset(embT[:], 0.0)
    nc.vector.memset(qblock[:], 0.0)
    nc.vector.memset(vcol[:], 0.0)

    def ln(z, g_ap, bt_ap, want_bf, tapn=None):
        comb = ap_.tile([128, 4], F32, tag="lncomb")
        nc.vector.tensor_reduce(comb[:, 0:2], z[:].rearrange("p c b -> p b c"), AX, ALU.add)
        sq = ap_.tile([128, NCK, B], F32, tag="lnsq")
        TT(sq[:], z[:], z[:], ALU.mult)
        nc.vector.tensor_reduce(comb[:, 2:4], sq[:].rearrange("p c b -> p b c"), AX, ALU.add)
        st_ps = pp.tile([128, 4], F32, tag="ps")
        nc.tensor.matmul(st_ps[:], ones_f32[:], comb[:], start=True, stop=True)
        t = ap_.tile([128, 4], F32, tag="lnt")
        nc.vector.tensor_scalar_mul(t[:], st_ps[:], 1.0 / D)
        mu2 = ap_.tile([128, 2], F32, tag="lnmu2")
        TT(mu2[:], t[:, 0:2], t[:, 0:2], ALU.mult)
        vare = ap_.tile([128, 2], F32, tag="lnvar")
        nc.vector.scalar_tensor_tensor(vare[:], t[:, 2:4], EPS, mu2[:], ALU.add, ALU.subtract)
        rstd = ap_.tile([128, 2], F32, tag="lnrstd")
        nc.vector.reciprocal(rstd[:], vare[:])
        nc.scalar.activation(rstd[:], rstd[:], ACTF.Sqrt)
        xh = ap_.tile([128, NCK, B], F32, tag="lnxh")
        TT(xh[:], z[:], t[:, 0:2].unsqueeze(1).broadcast_to((128, NCK, B)), ALU.subtract)
        TT(xh[:], xh[:], rstd[:].unsqueeze(1).broadcast_to((128, NCK, B)), ALU.mult)
        TT(xh[:], xh[:], g_ap, ALU.mult)
        x = ap_.tile([128, NCK, B], F32, tag="lnx")
        TT(x[:], xh[:], bt_ap, ALU.add)
        if tapn and tapn in taps:
            dma(taps[tapn][:], x[:])
        if want_bf:
            xbf = ap_.tile([128, NCK, B], BF16, tag="lnxb")
            CP(xbf[:], x[:])
            return x, xbf
        return x, None

    # ================= preamble ============================================
    with tc.tile_pool(name="pre", bufs=1) as prep, \
         tc.tile_pool(name="preps", bufs=3, space="PSUM") as preps:
        w_cav = load(prep, "w_cav"); w_cao = load(prep, "w_cao")
        w_adain = load(prep, "w_adain")
        b_cav = load(prep, "b_cav"); b_cao = load(prep, "b_cao")
        b_adain = load(prep, "b_adain")

        cc = prep.tile([128, D], F32, tag="cc")
        dma(cc[:], ins["content_code"].rearrange("b t d -> (b t) d"))
        st = prep.tile([B, D], F32, tag="st")
        dma(st[:], ins["style_code"][:])
        ist = prep.tile([B, M], F32, tag="ist")
        dma(ist[:], ins["init_state"][:])

        # content -> ccT [128, (4c, 2b, 64t)]
        ccT = prep.tile([128, NCK, B, T], F32, tag="ccT")
        for c in range(NCK):
            tp = preps.tile([128, 128], F32, tag="pps")
            nc.tensor.transpose(tp[:], cc[:, c * 128:(c + 1) * 128], ident_f32[:])
            CP(ccT[:, c, :, :], tp[:].rearrange("p (b t) -> p b t", b=B))

        # AdaIN stats over t (per (d-partition, c, b))
        mu = prep.tile([128, NCK, B], F32, tag="mu")
        nc.vector.tensor_reduce(mu[:], ccT[:], AX, ALU.add)
        sq = prep.tile([128, NCK, B, T], F32, tag="sqq")
        TT(sq[:], ccT[:], ccT[:], ALU.mult)
        s2 = prep.tile([128, NCK, B], F32, tag="s2")
        nc.vector.tensor_reduce(s2[:], sq[:], AX, ALU.add)
        nc.vector.tensor_scalar_mul(mu[:], mu[:], 1.0 / T)
        nc.vector.tensor_scalar_mul(s2[:], s2[:], 1.0 / T)
        mu2 = prep.tile([128, NCK, B], F32, tag="mu2")
        TT(mu2[:], mu[:], mu[:], ALU.mult)
        var = prep.tile([128, NCK, B], F32, tag="var")
        nc.vector.scalar_tensor_tensor(var[:], s2[:], EPS, mu2[:], ALU.add, ALU.subtract)
        rstd = prep.tile([128, NCK, B], F32, tag="rstd")
        nc.vector.reciprocal(rstd[:], var[:])
        nc.scalar.activation(rstd[:], rstd[:], ACTF.Sqrt)

        # styleT [128, (4c, 2b)]
        styT = prep.tile([128, NCK, B], F32, tag="styT")
        for c in range(NCK):
            tp = preps.tile([128, B], F32, tag="pps")
            nc.tensor.transpose(tp[:], st[:, c * 128:(c + 1) * 128], ident_f32[0:B, 0:B])
            CP(styT[:, c, :], tp[:])
        styb = prep.tile([128, NCK, B], BF16, tag="styb")
        CP(styb[:], styT[:])

        gd_ps = preps.tile([128, 8, B], F32, tag="pps")
        for mc in range(8):
            for kc in range(NCK):
                nc.tensor.matmul(gd_ps[:, mc, :], w_adain[:, kc, mc * 128:(mc + 1) * 128],
                                 styb[:, kc, :], start=(kc == 0), stop=(kc == NCK - 1))
        gd = prep.tile([128, 8, B], F32, tag="gdsb")
        TT(gd[:], gd_ps[:], b_adain[:], ALU.add)

        memb = prep.tile([128, NCK, B, T], BF16, tag="memb")
        tmpm = prep.tile([128, NCK, B, T], F32, tag="tmpm")
        TT(tmpm[:], ccT[:], mu[:].broadcast_to((128, NCK, B, T)), ALU.subtract)
        TT(tmpm[:], tmpm[:], rstd[:].broadcast_to((128, NCK, B, T)), ALU.mult)
        TT(tmpm[:], tmpm[:], gd[:, 0:NCK, :].broadcast_to((128, NCK, B, T)), ALU.mult)
        TT(tmpm[:], tmpm[:], gd[:, NCK:8, :].broadcast_to((128, NCK, B, T)), ALU.add)
        CP(memb[:], tmpm[:])
        if "memory" in taps:
            dma(taps["memory"][:], tmpm[:])

        for l in range(L):
            cav_ps = preps.tile([128, NCK, B * T], F32, tag="pps")
            for mc in range(NCK):
                for kc in range(NCK):
                    nc.tensor.matmul(cav_ps[:, mc, :], w_cav[:, l, kc, mc * 128:(mc + 1) * 128],
                                     memb[:, kc, :, :].rearrange("p b t -> p (b t)"),
                                     start=(kc == 0), stop=(kc == NCK - 1))
            cavb = prep.tile([128, NCK, B, T], BF16, tag="cavb")
            TT(cavb[:], cav_ps[:].rearrange("p m (b t) -> p m b t", b=B),
               b_cav[:, l, :, :].broadcast_to((128, NCK, B, T)), ALU.add)
            cao_ps = preps.tile([128, NCK, B * T], F32, tag="pps")
            for mc in range(NCK):
                for kc in range(NCK):
                    nc.tensor.matmul(cao_ps[:, mc, :], w_cao[:, l, kc, mc * 128:(mc + 1) * 128],
                                     cavb[:, kc, :, :].rearrange("p b t -> p (b t)"),
                                     start=(kc == 0), stop=(kc == NCK - 1))
            TT(ca_addT[:, l, :, :, :], cao_ps[:].rearrange("p m (b t) -> p m b t", b=B),
               b_cao[:, l, :, :].broadcast_to((128, NCK, B, T)), ALU.add)

        # emb0
        ib_ps = preps.tile([64, B], F32, tag="pps")
        nc.tensor.transpose(ib_ps[:], ist[:], ident_f32[0:B, 0:B])
        istb = prep.tile([64, B], BF16, tag="istb")
        CP(istb[:], ib_ps[:])
        e_ps = preps.tile([128, NCK, B], F32, tag="pps")
        for mc in range(NCK):
            nc.tensor.matmul(e_ps[:, mc, :], w_mm[:, mc * 128:(mc + 1) * 128], istb[:],
                             start=True, stop=True)
        TT(embT[:, :, :, 0], e_ps[:], b_mm[:], ALU.add)

    # main weights / psum pools (opened after the preamble pools are freed)
    wp = ctx.enter_context(tc.tile_pool(name="weights", bufs=1))
    pp = ctx.enter_context(tc.tile_pool(name="ps", bufs=5, space="PSUM"))
    vp = ctx.enter_context(tc.tile_pool(name="vps", bufs=1, space="PSUM"))
    V_ps = []
    for l in range(L):
        vtile = vp.tile([128, 512], F32, tag=f"vps{l}", name=f"vps{l}")
        V_ps.append(vtile)
    # zero-init V psum accumulators (vcol is all-zero here)
    for l in range(L):
        for c in range(NCK):
            for b in range(B):
                nc.tensor.matmul(V_ps[l][b * 64:(b + 1) * 64, c * 128:(c + 1) * 128],
                                 vcol[:, c, b, :], ident_bf[:],
                                 start=True, stop=True, skip_group_check=True)
    w_qkv = load(wp, "w_qkv"); w_out = load(wp, "w_out")
    w_ff1 = load(wp, "w_ff1"); w_ff2 = load(wp, "w_ff2")
    w_mmr = load(wp, "w_mmr")

    # ================= decode loop =========================================
    def step(i):
        dyn = not isinstance(i, int)

        def tap(name, ap, l=None):
            if not dyn and i == tap_i and (l is None or l == tap_l) and name in taps:
                nc.gpsimd.dma_start(taps[name][:], ap)

        x0 = ap_.tile([128, NCK, B], F32, tag="x0")
        TT(x0[:], embT[:, :, :, bass.ds(i, 1)].squeeze(),
           pe_t[:, :, bass.ds(i, 1)].broadcast_to((128, NCK, B)), ALU.add)
        xb = ap_.tile([128, NCK, B], BF16, tag="xb0")
        CP(xb[:], x0[:])
        x_res = x0
        tap("x0", x0[:])

        for l in range(L):
            # ---- QKV
            qkv_ps = pp.tile([128, 12, B], F32, tag="ps")
            for mc in range(12):
                for kc in range(NCK):
                    nc.tensor.matmul(qkv_ps[:, mc, :], w_qkv[:, l, kc, mc * 128:(mc + 1) * 128],
                                     xb[:, kc, :], start=(kc == 0), stop=(kc == NCK - 1))
            qkvb = ap_.tile([128, 12, B], BF16, tag="qkvb")
            TT(qkvb[:], qkv_ps[:], b_qkv[:, l, :, :], ALU.add)
            tap("qkvb", qkvb[:], l)

            # ---- q -> qblock (scaled); flat offsets 18*(2c+b) + hpar
            nc.vector.tensor_scalar_mul(
                qblock[0:64, 0::18].rearrange("p (c b) -> p c b", c=NCK),
                qkvb[0:64, 0:NCK, :], 1.0 / np.sqrt(HD))
            nc.vector.tensor_scalar_mul(
                qblock[64:128, 1::18].rearrange("p (c b) -> p c b", c=NCK),
                qkvb[64:128, 0:NCK, :], 1.0 / np.sqrt(HD))

            # ---- caches
            CP(KT[:, l, :, :, bass.ds(i, 1)].squeeze(), qkvb[:, 4:8, :])
            CP(vcol[:, :, :, bass.ds(i, 1)].squeeze(), qkvb[:, 8:12, :])

            # ---- V row-major accumulate + SBUF copy
            for c in range(NCK):
                for b in range(B):
                    nc.tensor.matmul(V_ps[l][b * 64:(b + 1) * 64, c * 128:(c + 1) * 128],
                                     vcol[:, c, b, :], ident_bf[:],
                                     start=False, stop=True, skip_group_check=True)
            CP(V_row[:, l, :], V_ps[l][:])

            # ---- scores
            sc_ps = pp.tile([16, T], F32, tag="ps")
            for c in range(NCK):
                for b in range(B):
                    e = 2 * c + b
                    nc.tensor.matmul(sc_ps[:], qblock[:, e * 16:(e + 1) * 16],
                                     KT[:, l, c, b, :], start=(e == 0), stop=(e == 7))
            s_sb = ap_.tile([16, T], F32, tag="s_sb")
            TT(s_sb[:], sc_ps[:], maskt[:, bass.ds(i, 1), :].squeeze(), ALU.add)
            tap("scores", s_sb[:], l)

            # ---- softmax (scores bounded; skip max-subtract)
            e_sb = ap_.tile([16, T], BF16, tag="e_sb")
            S = ap_.tile([16, 1], F32, tag="S")
            nc.scalar.activation(e_sb[:], s_sb[:], ACTF.Exp, accum_out=S[:])
            Sinv = ap_.tile([16, 1], F32, tag="Sinv")
            nc.vector.reciprocal(Sinv[:], S[:])
            p_sb = ap_.tile([16, T], BF16, tag="p_sb")
            nc.vector.tensor_scalar_mul(p_sb[:], e_sb[:], Sinv[:])

            # ---- pT duplicated on both partition halves
            pT_ps = pp.tile([128, 16], BF16, tag="ps")
            nc.tensor.transpose(pT_ps[0:64, :], p_sb[:], ident_bf[0:16, 0:16])
            nc.tensor.transpose(pT_ps[64:128, :], p_sb[:], ident_bf[0:16, 0:16],
                                tile_position=(0, 64))
            pTs = ap_.tile([128, 16], BF16, tag="pTs")
            CP(pTs[:], pT_ps[:])

            # ---- o matmuls -> oT [128, (4c, 2b)]
            oT_ps = pp.tile([128, NCK, B], F32, tag="ps")
            for h in range(H):
                c, hp = h // 2, h % 2
                for b in range(B):
                    s = slot_of(b, h)
                    nc.tensor.matmul(
                        oT_ps[hp * 64:(hp + 1) * 64, c, b:b + 1],
                        V_row[b * 64:(b + 1) * 64, l, h * 64:(h + 1) * 64],
                        pTs[b * 64:(b + 1) * 64, s:s + 1],
                        start=True, stop=True, tile_position=(b * 64, hp * 64))
            oTs = ap_.tile([128, NCK, B], BF16, tag="oTs")
            CP(oTs[:], oT_ps[:])
            tap("oTs", oTs[:], l)

            # ---- out projection + residual + LN1
            pr_ps = pp.tile([128, NCK, B], F32, tag="ps")
            for mc in range(NCK):
                for kc in range(NCK):
                    nc.tensor.matmul(pr_ps[:, mc, :], w_out[:, l, kc, mc * 128:(mc + 1) * 128],
                                     oTs[:, kc, :], start=(kc == 0), stop=(kc == NCK - 1))
            z = ap_.tile([128, NCK, B], F32, tag="z1")
            TT(z[:], pr_ps[:], b_out[:, l, :, :], ALU.add)
            TT(z[:], z[:], x_res[:], ALU.add)
            x1, _ = ln(z, g_ln[:, l, 0], bt_ln[:, l, 0], want_bf=False,
                       tapn="x1" if (not dyn and i == tap_i and l == tap_l) else None)

            # ---- cross-attn constant + LN2
            z2 = ap_.tile([128, NCK, B], F32, tag="z2")
            TT(z2[:], x1[:], ca_addT[:, l, :, :, bass.ds(i, 1)].squeeze(), ALU.add)
            x2, x2b = ln(z2, g_ln[:, l, 1], bt_ln[:, l, 1], want_bf=True)

            # ---- FFN + LN3
            ff_ps = pp.tile([128, NF, B], F32, tag="ps")
            for mc in range(NF):
                for kc in range(NCK):
                    nc.tensor.matmul(ff_ps[:, mc, :], w_ff1[:, l, kc, mc * 128:(mc + 1) * 128],
                                     x2b[:, kc, :], start=(kc == 0), stop=(kc == NCK - 1))
            hsum = ap_.tile([128, NF, B], F32, tag="hsum")
            TT(hsum[:], ff_ps[:], b_ff1[:, l, :, :], ALU.add)
            hb = ap_.tile([128, NF, B], BF16, tag="hb")
            nc.vector.tensor_scalar_max(hb[:], hsum[:], 0.0)

            f2_ps = pp.tile([128, NCK, B], F32, tag="ps")
            for mc in range(NCK):
                for kc in range(NF):
                    nc.tensor.matmul(f2_ps[:, mc, :], w_ff2[:, l, kc, mc * 128:(mc + 1) * 128],
                                     hb[:, kc, :], start=(kc == 0), stop=(kc == NF - 1))
            z3 = ap_.tile([128, NCK, B], F32, tag="z3")
            TT(z3[:], f2_ps[:], b_ff2[:, l, :, :], ALU.add)
            TT(z3[:], z3[:], x2[:], ALU.add)
            x3, x3b = ln(z3, g_ln[:, l, 2], bt_ln[:, l, 2], want_bf=True,
                         tapn="x3" if (not dyn and i == tap_i and l == tap_l) else None)
            x_res = x3
            xb = x3b

        # clear vcol column (holds layer-2's v)
        nc.vector.memset(vcol[:, :, :, bass.ds(i, 1)].squeeze(), 0.0)

        # ---- output row + next emb
        r_ps = pp.tile([64, B], F32, tag="ps")
        for kc in range(NCK):
            nc.tensor.matmul(r_ps[:], w_mmr[:, kc, :], xb[:, kc, :],
                             start=(kc == 0), stop=(kc == NCK - 1))
        nc.vector.tensor_scalar_add(out_sb[:, :, bass.ds(i, 1)].squeeze(), r_ps[:], b_mmr[:, 0:1])
        rowb = ap_.tile([64, B], BF16, tag="rowb")
        nc.vector.tensor_scalar_add(rowb[:], r_ps[:], b_mmr[:, 0:1])
        e_ps = pp.tile([128, NCK, B], F32, tag="ps")
        for mc in range(NCK):
            nc.tensor.matmul(e_ps[:, mc, :], w_mm[:, mc * 128:(mc + 1) * 128], rowb[:],
                             start=True, stop=True)
        TT(embT[:, :, :, bass.ds(i + 1, 1)].squeeze(), e_ps[:], b_mm[:], ALU.add)

    if dyn_loop:
        with tc.For_i(0, n_steps, 1, hint_engines=(mybir.EngineType.PE,), staggered_reset=staggered) as i:
            step(i)
    else:
        for i in range(n_steps):
            step(i)

    # ---- final output
    fo_ps = pp.tile([128, 64], F32, tag="ps")
    nc.tensor.transpose(fo_ps[:], out_sb[:].rearrange("p b t -> p (b t)"),
                        ident_f32[0:64, 0:64])
    fo = ap_.tile([128, 64], F32, tag="fo")
    CP(fo[:], fo_ps[:])
    dma(outs["out"].rearrange("b t m -> (b t) m"), fo[:])

    ctx.close()


# ===================================================================== runner
_CACHE = {}


def _build_and_compile():
    if "nc" in _CACHE:
        return
    import concourse.tile as _tile
    from concourse import bacc as _bacc
    nc = _bacc.Bacc("TRN2", target_bir_lowering=False, debug=False)
    ins, outs = {}, {}
    for name, (shape, dt) in input_specs().items():
        ins[name] = nc.dram_tensor(name, list(shape), mybir.dt.from_np(np.dtype(dt)),
                                   kind="ExternalInput").ap()
    outs["out"] = nc.dram_tensor("out", [B, T, M], mybir.dt.float32,
                                 kind="ExternalOutput").ap()
    with _tile.TileContext(nc) as tc:
        build(tc, ins, outs, n_steps=T, dyn_loop=True)
    nc.compile()
    _CACHE["nc"] = nc


def kernel(**inputs):
    """Full (unsharded) inputs -> full output [B, T, M] float32."""
    from concourse.bass_utils import run_bass_kernel_spmd
    _build_and_compile()
    dev_ins = prep_inputs(inputs)
    res = run_bass_kernel_spmd(_CACHE["nc"], [dev_ins], core_ids=[0])
    return np.ascontiguousarray(res.results[0]["out"].astype(np.float32))



# revision 2
# speedup vs baseline: 1.0324x; 1.0324x over previous
"""Kernel for nn_DecoderAutoregAdaIN on TRN2 (single core, KV-cache decode).

Key changes vs baseline:
  - ACT engine uses ONLY the natural_log_exp table set (Exp for softmax,
    Ln+Exp for rstd = exp(-0.5 ln(var+eps))) -> no per-layer ACT table
    reloads (~2.7us each).
  - LN3[l-1] folded into sa_w[l] (l>=1) / LN3[2] into fused W_emb; LN2[l]
    folded into ff1[l]. Stats (mean/rstd) compute concurrently with the
    big weight-streaming matmuls; only a 2-op correction sits on the
    critical path.
  - mmr deferred: x3-final stored per step; all 64 output rows computed in
    one batched matmul at the end.
  - emb recurrence via fused W_emb = mm_w @ mmr_w (one matmul stage).

Layouts identical to baseline (partition-major activations; feature
f = c*128 + p, head h = 2c + (p>=64)).
"""
from contextlib import ExitStack
import numpy as np
import ml_dtypes

import concourse.bass as bass
from concourse import mybir
from concourse.alu_op_type import AluOpType as ALU

F32 = mybir.dt.float32
BF16 = mybir.dt.bfloat16
AX = mybir.AxisListType.X
ACTF = mybir.ActivationFunctionType

B, T, D, M, H, L, DFF, PERIOD = 2, 64, 512, 64, 8, 3, 2048, 30
HD = D // H
EPS = 1e-5
NCK = 4          # feature chunks of 128
NF = DFF // 128  # 16


def slot_of(b, h):
    # block-c aligned: within score-stationary block c = h//2, live columns are
    # 8*hp + 2*c + b at flat column 18*c + 8*hp + b (hp = h%2)
    return 8 * (h % 2) + 2 * (h // 2) + b


# ---------------------------------------------------------------- host prep
def _slopes(n):
    start = 2.0 ** (-(2.0 ** -(np.log2(n) - 3)))
    return np.array([start * start ** i for i in range(n)], dtype=np.float32)


def _pe_mask():
    pos = np.arange(PERIOD)[:, None].astype(np.float32)
    div = np.exp(np.arange(0, D, 2).astype(np.float32) * (-np.log(10000.0) / D))
    pe = np.zeros((PERIOD, D), np.float32)
    pe[:, 0::2] = np.sin(pos * div)
    pe[:, 1::2] = np.cos(pos * div)
    pe_full = np.tile(pe, (T // PERIOD + 1, 1))[:T]
    ii = np.arange(T)[:, None]
    jj = np.arange(T)[None, :]
    bias = -((ii - jj) // PERIOD).astype(np.float32)
    alibi = _slopes(H)[:, None, None] * np.where(jj <= ii, bias, 0.0)
    self_mask = np.where(jj <= ii, alibi, -1e9).astype(np.float32)  # [H,T,T]
    return pe_full, self_mask


def _wtiles(w_t, n_kc):
    """w_t [K, Mo] -> [128, n_kc, Mo]; lhsT tile (kc, mc) = arr[:, kc, mc*128:(mc+1)*128]."""
    K, Mo = w_t.shape
    assert K == n_kc * 128
    return np.ascontiguousarray(w_t.reshape(n_kc, 128, Mo).transpose(1, 0, 2))


def _bf(x):
    return np.ascontiguousarray(np.asarray(x).astype(ml_dtypes.bfloat16))


def _dup_b(x):  # append duplicated b axis of size B
    return np.ascontiguousarray(np.repeat(np.asarray(x, np.float32)[..., None], B, axis=-1))


def _pm(v, n):  # [n*128] -> [128, n] partition-major
    return np.ascontiguousarray(np.asarray(v, np.float32).reshape(n, 128).T)


def prep_inputs(inp):
    inp = {k: np.asarray(v, np.float32) for k, v in inp.items()}
    pe_full, self_mask = _pe_mask()
    out = {}
    g = inp["ln_g"]   # [L, 3, D]
    bt = inp["ln_b"]

    # folded QKV (LN3[l-1] for l>=1)
    qkv_w, qkv_u, qkv_c = [], [], []
    for l in range(L):
        W, bW = inp["sa_w"][l], inp["sa_b"][l]
        if l == 0:
            qkv_w.append(W)
            qkv_u.append(np.zeros(3 * D, np.float32))
            qkv_c.append(bW)
        else:
            gg, bb = g[l - 1, 2], bt[l - 1, 2]
            qkv_w.append(W * gg[None, :])
            qkv_u.append(W @ gg)
            qkv_c.append(W @ bb + bW)
    out["w_qkv"] = _bf(np.stack([_wtiles(qkv_w[l].T, NCK) for l in range(L)], axis=1))
    out["u_qkv"] = _dup_b(np.stack([qkv_u[l].reshape(12, 128).T for l in range(L)], axis=1))
    out["c_qkv"] = _dup_b(np.stack([qkv_c[l].reshape(12, 128).T for l in range(L)], axis=1))

    # folded ff1 (LN2[l])
    ff1_w, ff1_u, ff1_c = [], [], []
    for l in range(L):
        W, bW = inp["ff1_w"][l], inp["ff1_b"][l]
        gg, bb = g[l, 1], bt[l, 1]
        ff1_w.append(W * gg[None, :])
        ff1_u.append(W @ gg)
        ff1_c.append(W @ bb + bW)
    out["w_ff1"] = _bf(np.stack([_wtiles(ff1_w[l].T, NCK) for l in range(L)], axis=1))
    out["u_ff1"] = _dup_b(np.stack([ff1_u[l].reshape(NF, 128).T for l in range(L)], axis=1))
    out["c_ff1"] = _dup_b(np.stack([ff1_c[l].reshape(NF, 128).T for l in range(L)], axis=1))

    # fused emb map with LN3[2] folded; c includes pe[t] per target position
    W_emb0 = inp["mm_w"] @ inp["mmr_w"]            # [D, D]
    b_emb0 = inp["mm_w"] @ inp["mmr_b"] + inp["mm_b"]
    gg, bb = g[2, 2], bt[2, 2]
    out["w_emb"] = _bf(_wtiles((W_emb0 * gg[None, :]).T, NCK))       # [128, 4, 512]
    out["u_emb"] = _dup_b(_pm(W_emb0 @ gg, NCK))                      # [128, 4, B]
    c_emb = W_emb0 @ bb + b_emb0                                      # [D]
    c_embpe = np.zeros((T + 1, D), np.float32)                        # col t = for x0 of step t
    c_embpe[:T] = c_emb[None, :] + pe_full
    out["c_embpe"] = np.ascontiguousarray(
        c_embpe.T.reshape(NCK, 128, T + 1).transpose(1, 0, 2))        # [128, 4, T+1]

    out["w_out"] = _bf(np.stack([_wtiles(inp["sa_o_w"][l].T, NCK) for l in range(L)], axis=1))
    out["w_ff2"] = _bf(np.stack([_wtiles(inp["ff2_w"][l].T, NF) for l in range(L)], axis=1))
    out["w_cav"] = _bf(np.stack([_wtiles(inp["ca_w"][l][2 * D:].T, NCK) for l in range(L)], axis=1))
    out["w_cao"] = _bf(np.stack([_wtiles(inp["ca_o_w"][l].T, NCK) for l in range(L)], axis=1))
    out["w_adain"] = _bf(_wtiles(inp["adain_w"].T, NCK))      # [128, 4, 1024]
    out["w_mm"] = _bf(inp["mm_w"].T)                          # [64, 512]
    out["w_mmr"] = _bf(_wtiles(inp["mmr_w"].T, NCK))          # [128, 4, 64]

    out["b_out"] = _dup_b(np.stack([_pm(inp["sa_o_b"][l], NCK) for l in range(L)], axis=1))
    out["b_ff2"] = _dup_b(np.stack([_pm(inp["ff2_b"][l], NCK) for l in range(L)], axis=1))
    out["b_cav"] = _dup_b(np.stack([_pm(inp["ca_b"][l][2 * D:], NCK) for l in range(L)], axis=1))
    out["b_cao"] = _dup_b(np.stack([_pm(inp["ca_o_b"][l], NCK) for l in range(L)], axis=1))
    out["b_adain"] = _dup_b(inp["adain_b"].reshape(8, 128).T)
    out["b_mm"] = _dup_b(_pm(inp["mm_b"], NCK))
    out["b_mmr"] = np.ascontiguousarray(np.repeat(inp["mmr_b"][:, None], B, axis=1))

    gs = np.stack([np.stack([_pm(g[l, j], NCK) for j in range(3)]) for l in range(L)])
    bs = np.stack([np.stack([_pm(bt[l, j], NCK) for j in range(3)]) for l in range(L)])
    out["g_ln"] = _dup_b(gs.transpose(2, 0, 1, 3))   # [128, L, 3, 4, B]
    out["bt_ln"] = _dup_b(bs.transpose(2, 0, 1, 3))

    out["pe_t"] = np.ascontiguousarray(pe_full.T.reshape(NCK, 128, T).transpose(1, 0, 2))
    mask = np.full((16, T, B, T), -1e9, np.float32)
    for b in range(B):
        for h in range(H):
            mask[slot_of(b, h), :, b, :] = self_mask[h]
    out["maskt"] = _bf(mask)
    # combined residual constants
    bxr = np.zeros((128, L, NCK, B), np.float32)
    for l in range(1, L):
        bxr[:, l] = (_pm(bt[l - 1, 2], NCK) + _pm(inp["sa_o_b"][l], NCK))[:, :, None]
    out["bxr"] = bxr
    out["bt2x"] = _dup_b(np.stack([_pm(bt[l, 1], NCK) + _pm(inp["ff2_b"][l], NCK)
                                   for l in range(L)], axis=1))
    out["ident_bf"] = _bf(np.eye(128, dtype=np.float32))
    out["ident_f32"] = np.eye(128, dtype=np.float32)
    out["ones_scl"] = np.full((128, 128), 1.0 / D, np.float32)

    out["content_code"] = np.ascontiguousarray(inp["content_code"])
    out["style_code"] = np.ascontiguousarray(inp["style_code"])
    out["init_state"] = np.ascontiguousarray(inp["init_state"])
    return out


def input_specs():
    bf, f32 = ml_dtypes.bfloat16, np.float32
    return {
        "w_qkv": ((128, L, NCK, 3 * D), bf), "w_out": ((128, L, NCK, D), bf),
        "w_ff1": ((128, L, NCK, DFF), bf), "w_ff2": ((128, L, NF, D), bf),
        "w_cav": ((128, L, NCK, D), bf), "w_cao": ((128, L, NCK, D), bf),
        "w_adain": ((128, NCK, 2 * D), bf), "w_mm": ((64, D), bf),
        "w_mmr": ((128, NCK, M), bf), "w_emb": ((128, NCK, D), bf),
        "u_qkv": ((128, L, 12, B), f32), "c_qkv": ((128, L, 12, B), f32),
        "u_ff1": ((128, L, NF, B), f32), "c_ff1": ((128, L, NF, B), f32),
        "u_emb": ((128, NCK, B), f32), "c_embpe": ((128, NCK, T + 1), f32),
        "b_out": ((128, L, NCK, B), f32), "b_ff2": ((128, L, NCK, B), f32),
        "b_cav": ((128, L, NCK, B), f32), "b_cao": ((128, L, NCK, B), f32),
        "b_adain": ((128, 8, B), f32), "b_mm": ((128, NCK, B), f32),
        "b_mmr": ((64, B), f32),
        "g_ln": ((128, L, 3, NCK, B), f32), "bt_ln": ((128, L, 3, NCK, B), f32),
        "bxr": ((128, L, NCK, B), f32), "bt2x": ((128, L, NCK, B), f32),
        "pe_t": ((128, NCK, T), f32), "maskt": ((16, T, B, T), bf),
        "ident_bf": ((128, 128), bf), "ident_f32": ((128, 128), f32),
        "ones_scl": ((128, 128), f32),
        "content_code": ((B, T, D), f32), "style_code": ((B, D), f32),
        "init_state": ((B, M), f32),
    }


# ---------------------------------------------------------------- builder
def build(tc, ins, outs, n_steps=T, loop_mode="dyn", taps=None, tap_at=(0, 0)):
    nc = tc.nc
    ctx = ExitStack()
    taps = taps or {}
    tap_i, tap_l = tap_at

    cp = ctx.enter_context(tc.tile_pool(name="consts", bufs=1))
    sp = ctx.enter_context(tc.tile_pool(name="state", bufs=1))
    ap_ = ctx.enter_context(tc.tile_pool(name="act", bufs=2))

    dma = nc.sync.dma_start
    TT = nc.vector.tensor_tensor
    STT = nc.vector.scalar_tensor_tensor
    CP = nc.vector.tensor_copy
    ACT = nc.scalar.activation

    def load(pool, name):
        src = ins[name]
        t = pool.tile(list(src.shape), src.dtype, tag=name)
        dma(t[:], src[:])
        return t

    w_mm = load(cp, "w_mm")
    u_qkv = load(cp, "u_qkv"); c_qkv = load(cp, "c_qkv")
    u_ff1 = load(cp, "u_ff1"); c_ff1 = load(cp, "c_ff1")
    u_emb = load(cp, "u_emb"); c_embpe = load(cp, "c_embpe")
    b_out = load(cp, "b_out"); b_ff2 = load(cp, "b_ff2")
    b_mm = load(cp, "b_mm"); b_mmr = load(cp, "b_mmr")
    g_ln = load(cp, "g_ln"); bt_ln = load(cp, "bt_ln")
    bxr = load(cp, "bxr"); bt2x = load(cp, "bt2x")
    pe_t = load(cp, "pe_t"); maskt = load(cp, "maskt")
    ident_bf = load(cp, "ident_bf"); ident_f32 = load(cp, "ident_f32")
    ones_scl = load(cp, "ones_scl")

    KT = sp.tile([128, L, NCK, B, T], BF16, tag="KT")
    V_row = sp.tile([128, L, D], BF16, tag="V_row")
    ca_addT = sp.tile([128, L, NCK, B, T], F32, tag="ca_addT")
    xfin = sp.tile([128, NCK, B, T], BF16, tag="xfin")
    qblock = sp.tile([128, 80], BF16, tag="qblock")
    vcol = sp.tile([128, NCK, B, T], BF16, tag="vcol")
    x0_t = sp.tile([128, NCK, B], F32, tag="x0")
    x0b_t = sp.tile([128, NCK, B], BF16, tag="x0b")

    nc.vector.memset(KT[:], 0.0)
    nc.vector.memset(qblock[:], 0.0)
    nc.vector.memset(vcol[:], 0.0)
    nc.vector.memset(xfin[:], 0.0)

    # ================= preamble ============================================
    with tc.tile_pool(name="pre", bufs=1) as prep, \
         tc.tile_pool(name="preps", bufs=3, space="PSUM") as preps:
        w_cav = load(prep, "w_cav"); w_cao = load(prep, "w_cao")
        w_adain = load(prep, "w_adain")
        b_cav = load(prep, "b_cav"); b_cao = load(prep, "b_cao")
        b_adain = load(prep, "b_adain")

        cc = prep.tile([128, D], F32, tag="cc")
        dma(cc[:], ins["content_code"].rearrange("b t d -> (b t) d"))
        st = prep.tile([B, D], F32, tag="st")
        dma(st[:], ins["style_code"][:])
        ist = prep.tile([B, M], F32, tag="ist")
        dma(ist[:], ins["init_state"][:])

        ccT = prep.tile([128, NCK, B, T], F32, tag="ccT")
        for c in range(NCK):
            tp = preps.tile([128, 128], F32, tag="pps")
            nc.tensor.transpose(tp[:], cc[:, c * 128:(c + 1) * 128], ident_f32[:])
            CP(ccT[:, c, :, :], tp[:].rearrange("p (b t) -> p b t", b=B))

        mu = prep.tile([128, NCK, B], F32, tag="mu")
        nc.vector.tensor_reduce(mu[:], ccT[:], AX, ALU.add)
        sq = prep.tile([128, NCK, B, T], F32, tag="sqq")
        TT(sq[:], ccT[:], ccT[:], ALU.mult)
        s2 = prep.tile([128, NCK, B], F32, tag="s2")
        nc.vector.tensor_reduce(s2[:], sq[:], AX, ALU.add)
        nc.vector.tensor_scalar_mul(mu[:], mu[:], 1.0 / T)
        nc.vector.tensor_scalar_mul(s2[:], s2[:], 1.0 / T)
        mu2 = prep.tile([128, NCK, B], F32, tag="mu2")
        TT(mu2[:], mu[:], mu[:], ALU.mult)
        var = prep.tile([128, NCK, B], F32, tag="var")
        nc.vector.scalar_tensor_tensor(var[:], s2[:], EPS, mu2[:], ALU.add, ALU.subtract)
        lnv = prep.tile([128, NCK, B], F32, tag="lnv")
        ACT(lnv[:], var[:], ACTF.Ln)
        rstd = prep.tile([128, NCK, B], F32, tag="rstd")
        ACT(rstd[:], lnv[:], ACTF.Exp, scale=-0.5)

        styT = prep.tile([128, NCK, B], F32, tag="styT")
        for c in range(NCK):
            tp = preps.tile([128, B], F32, tag="pps")
            nc.tensor.transpose(tp[:], st[:, c * 128:(c + 1) * 128], ident_f32[0:B, 0:B])
            CP(styT[:, c, :], tp[:])
        styb = prep.tile([128, NCK, B], BF16, tag="styb")
        CP(styb[:], styT[:])

        gd_ps = preps.tile([128, 8, B], F32, tag="pps")
        for mc in range(8):
            for kc in range(NCK):
                nc.tensor.matmul(gd_ps[:, mc, :], w_adain[:, kc, mc * 128:(mc + 1) * 128],
                                 styb[:, kc, :], start=(kc == 0), stop=(kc == NCK - 1))
        gd = prep.tile([128, 8, B], F32, tag="gdsb")
        TT(gd[:], gd_ps[:], b_adain[:], ALU.add)

        memb = prep.tile([128, NCK, B, T], BF16, tag="memb")
        tmpm = prep.tile([128, NCK, B, T], F32, tag="tmpm")
        TT(tmpm[:], ccT[:], mu[:].broadcast_to((128, NCK, B, T)), ALU.subtract)
        TT(tmpm[:], tmpm[:], rstd[:].broadcast_to((128, NCK, B, T)), ALU.mult)
        TT(tmpm[:], tmpm[:], gd[:, 0:NCK, :].broadcast_to((128, NCK, B, T)), ALU.mult)
        TT(tmpm[:], tmpm[:], gd[:, NCK:8, :].broadcast_to((128, NCK, B, T)), ALU.add)
        CP(memb[:], tmpm[:])
        if "memory" in taps:
            dma(taps["memory"][:], tmpm[:])

        for l in range(L):
            cav_ps = preps.tile([128, NCK, B * T], F32, tag="pps")
            for mc in range(NCK):
                for kc in range(NCK):
                    nc.tensor.matmul(cav_ps[:, mc, :], w_cav[:, l, kc, mc * 128:(mc + 1) * 128],
                                     memb[:, kc, :, :].rearrange("p b t -> p (b t)"),
                                     start=(kc == 0), stop=(kc == NCK - 1))
            cavb = prep.tile([128, NCK, B, T], BF16, tag="cavb")
            TT(cavb[:], cav_ps[:].rearrange("p m (b t) -> p m b t", b=B),
               b_cav[:, l, :, :].broadcast_to((128, NCK, B, T)), ALU.add)
            cao_ps = preps.tile([128, NCK, B * T], F32, tag="pps")
            for mc in range(NCK):
                for kc in range(NCK):
                    nc.tensor.matmul(cao_ps[:, mc, :], w_cao[:, l, kc, mc * 128:(mc + 1) * 128],
                                     cavb[:, kc, :, :].rearrange("p b t -> p (b t)"),
                                     start=(kc == 0), stop=(kc == NCK - 1))
            TT(ca_addT[:, l, :, :, :], cao_ps[:].rearrange("p m (b t) -> p m b t", b=B),
               b_cao[:, l, :, :].broadcast_to((128, NCK, B, T)), ALU.add)

        # x0 for step 0: mm(init) + b_mm + pe[0]
        ib_ps = preps.tile([64, B], F32, tag="pps")
        nc.tensor.transpose(ib_ps[:], ist[:], ident_f32[0:B, 0:B])
        istb = prep.tile([64, B], BF16, tag="istb")
        CP(istb[:], ib_ps[:])
        e_ps = preps.tile([128, NCK, B], F32, tag="pps")
        for mc in range(NCK):
            nc.tensor.matmul(e_ps[:, mc, :], w_mm[:, mc * 128:(mc + 1) * 128], istb[:],
                             start=True, stop=True)
        TT(x0_t[:], e_ps[:], b_mm[:], ALU.add)
        TT(x0_t[:], x0_t[:], pe_t[:, :, 0:1].broadcast_to((128, NCK, B)), ALU.add)
        CP(x0b_t[:], x0_t[:])

    # main weights / psum pools
    wp = ctx.enter_context(tc.tile_pool(name="weights", bufs=1))
    pp = ctx.enter_context(tc.tile_pool(name="ps", bufs=5, space="PSUM"))
    vp = ctx.enter_context(tc.tile_pool(name="vps", bufs=1, space="PSUM"))
    V_ps = []
    for l in range(L):
        vtile = vp.tile([128, 512], F32, tag=f"vps{l}", name=f"vps{l}")
        V_ps.append(vtile)
    for l in range(L):
        for c in range(NCK):
            nc.tensor.matmul(V_ps[l][:, c * 128:(c + 1) * 128],
                             vcol[:, c, :, :].rearrange("p b t -> p (b t)"),
                             ident_bf[:],
                             start=True, stop=True, skip_group_check=True)
    w_qkv = load(wp, "w_qkv"); w_out = load(wp, "w_out")
    w_ff1 = load(wp, "w_ff1"); w_ff2 = load(wp, "w_ff2")
    w_emb = load(wp, "w_emb"); w_mmr = load(wp, "w_mmr")

    # ---------------- helpers ---------------------------------------------
    def zs_tile(tag):
        """Holds [z, z^2] stacked on axis 1 so one reduce covers both."""
        return ap_.tile([128, 2, NCK, B], F32, tag=f"{tag}_zs", name=f"{tag}_zs")

    def stats_pre(zs, tag):
        """DVE-side: square + one merged reduce. zs[:, 0] must hold z."""
        TT(zs[:, 1], zs[:, 0], zs[:, 0], ALU.mult)
        comb = ap_.tile([128, 4], F32, tag=f"{tag}_comb")
        nc.vector.tensor_reduce(comb[:].rearrange("p (s b) -> p s b", s=2),
                                zs[:].rearrange("p s c b -> p s b c"), AX, ALU.add)
        return comb

    def stats_post(comb, tag, dve=False, st_region=None):
        """PE partition-sum + rstd. st_region: [128, 4] psum AP co-located in
        an already-allocated bank (avoids an extra pool allocation)."""
        if st_region is None:
            st_ps = pp.tile([128, 4], F32, tag="ps")
            st_ps = st_ps[:]
        else:
            st_ps = st_region
        nc.tensor.matmul(st_ps, ones_scl[:], comb[:], start=True, stop=True,
                         skip_group_check=True)
        t = ap_.tile([128, 4], F32, tag=f"{tag}_t")
        mu2 = ap_.tile([128, 2], F32, tag=f"{tag}_mu2")
        CP(t[:], st_ps)
        TT(mu2[:], t[:, 0:2], t[:, 0:2], ALU.mult)
        vare = ap_.tile([128, 2], F32, tag=f"{tag}_var")
        STT(vare[:], t[:, 2:4], EPS, mu2[:], ALU.add, ALU.subtract)
        lnv = ap_.tile([128, 2], F32, tag=f"{tag}_lnv")
        ACT(lnv[:], vare[:], ACTF.Ln)
        alpha = ap_.tile([128, 2], F32, tag=f"{tag}_al")
        ACT(alpha[:], lnv[:], ACTF.Exp, scale=-0.5)
        am = ap_.tile([128, 2], F32, tag=f"{tag}_am")
        TT(am[:], alpha[:], t[:, 0:2], ALU.mult)
        return alpha, am, t

    def materialize(z, alpha, mu_ap, l, j, tag, out_bf=False, bias_ap=None, out_ap=None):
        """x = (z - mu) * (g*alpha) + b   (4 TTs). bias_ap overrides b."""
        gsl = g_ln[:, l, j]
        A = ap_.tile([128, NCK, B], F32, tag=f"{tag}_A")
        TT(A[:], gsl, alpha[:].unsqueeze(1).broadcast_to((128, NCK, B)), ALU.mult)
        zc = ap_.tile([128, NCK, B], F32, tag=f"{tag}_zc")
        zap = z if isinstance(z, bass.AP) else z[:]
        TT(zc[:], zap, mu_ap.unsqueeze(1).broadcast_to((128, NCK, B)), ALU.subtract)
        m_ = ap_.tile([128, NCK, B], F32, tag=f"{tag}_m")
        TT(m_[:], zc[:], A[:], ALU.mult)
        if out_ap is not None:
            TT(out_ap, m_[:], bias_ap if bias_ap is not None else bt_ln[:, l, j], ALU.add)
            return out_ap, None
        x = ap_.tile([128, NCK, B], F32, tag=f"{tag}_x")
        TT(x[:], m_[:], bias_ap if bias_ap is not None else bt_ln[:, l, j], ALU.add)
        if out_bf:
            xb = ap_.tile([128, NCK, B], BF16, tag=f"{tag}_xb")
            CP(xb[:], x[:])
            return x, xb
        return x, None

    def correct(ps_ap, alpha, am, u_sl, c_sl, nm, tag, odt=BF16):
        """out = ps*alpha + (c - u*am)   [128, nm, B]."""
        t1 = ap_.tile([128, nm, B], F32, tag=f"{tag}_t1")
        TT(t1[:], u_sl, am[:].unsqueeze(1).broadcast_to((128, nm, B)), ALU.mult)
        t2 = ap_.tile([128, nm, B], F32, tag=f"{tag}_t2")
        TT(t2[:], c_sl, t1[:], ALU.subtract)
        ya = ap_.tile([128, nm, B], F32, tag=f"{tag}_ya")
        TT(ya[:], ps_ap, alpha[:].unsqueeze(1).broadcast_to((128, nm, B)), ALU.mult)
        o = ap_.tile([128, nm, B], odt, tag=f"{tag}_o")
        TT(o[:], ya[:], t2[:], ALU.add)
        return o

    # ================= decode loop =========================================
    def step(i):
        dyn = not isinstance(i, int)

        def tap(name, ap, l=None):
            if not dyn and i == tap_i and (l is None or l == tap_l) and name in taps:
                nc.gpsimd.dma_start(taps[name][:], ap)

        xb = x0b_t          # bf16 matmul input for layer 0
        z_prev = None       # z3 of previous layer (pre-LN3), fp32
        comb3 = None        # DVE-side stats reduces of z_prev

        for l in range(L):
            # ---- QKV (weights pre-folded with LN3[l-1] for l>=1)
            qkv_ps_t = pp.tile([128, 12, B], F32, tag="ps")
            qkv_ps = qkv_ps_t[:]
            for mc in range(12):
                for kc in range(NCK):
                    nc.tensor.matmul(qkv_ps[:, mc, :], w_qkv[:, l, kc, mc * 128:(mc + 1) * 128],
                                     xb[:, kc, :], start=(kc == 0), stop=(kc == NCK - 1))
            # bca for this layer's LN1->z2 fold (independent of anything running)
            bca = ap_.tile([128, NCK, B], F32, tag="bca")
            TT(bca[:], bt_ln[:, l, 0], ca_addT[:, l, :, :, bass.ds(i, 1)].squeeze(), ALU.add)
            if l == 0:
                qkvb = ap_.tile([128, 12, B], BF16, tag="qkvb")
                TT(qkvb[:], qkv_ps, c_qkv[:, l, :, :], ALU.add)
                bx0 = ap_.tile([128, NCK, B], F32, tag="bx0")
                TT(bx0[:], x0_t[:], b_out[:, 0, :, :], ALU.add)
                x_resc = bx0      # residual + proj bias, combined
            else:
                # stats3 of z_prev: PE partition-sum lands after the QKV stream
                a3, am3, t3 = stats_post(comb3, f"s3p_{l}")
                qkvb = correct(qkv_ps, a3, am3, u_qkv[:, l, :, :], c_qkv[:, l, :, :],
                               12, f"qc{l}")
                # x3[l-1] + b_out[l] combined (only used as z1 residual)
                x_resc, _ = materialize(z_prev, a3, t3[:, 0:2], l - 1, 2, f"x3m{l}",
                                        bias_ap=bxr[:, l, :, :])
            tap("qkvb", qkvb[:], l)

            # ---- q -> qblock (scaled); flat col = 18c + 8hp + b
            nc.vector.tensor_scalar_mul(
                qblock[0:64, 0:72].rearrange("p (c r) -> p c r", c=NCK)[:, :, 0:B],
                qkvb[0:64, 0:NCK, :], 1.0 / np.sqrt(HD))
            nc.vector.tensor_scalar_mul(
                qblock[64:128, 8:80].rearrange("p (c r) -> p c r", c=NCK)[:, :, 0:B],
                qkvb[64:128, 0:NCK, :], 1.0 / np.sqrt(HD))

            # ---- caches
            CP(KT[:, l, :, :, bass.ds(i, 1)].squeeze(), qkvb[:, 4:8, :])
            CP(vcol[:, :, :, bass.ds(i, 1)].squeeze(), qkvb[:, 8:12, :])

            # ---- V accumulate (PE, batched over b) + SBUF copy (ACT)
            for c in range(NCK):
                nc.tensor.matmul(V_ps[l][:, c * 128:(c + 1) * 128],
                                 vcol[:, c, :, :].rearrange("p b t -> p (b t)"),
                                 ident_bf[:],
                                 start=False, stop=True, skip_group_check=True)
            nc.scalar.copy(V_row[:, l, :], V_ps[l][:])

            # ---- scores, batched over b: [16, (b t)]; o2 shares the bank
            sc_ps_t = pp.tile([16, B * T], F32, tag="ps")
            sc_ps = sc_ps_t[:]
            for c in range(NCK):
                nc.tensor.matmul(sc_ps, qblock[:, c * 16:(c + 1) * 16],
                                 KT[:, l, c, :, :].rearrange("p b t -> p (b t)"),
                                 start=(c == 0), stop=(c == NCK - 1))
            s_sb = ap_.tile([16, B * T], F32, tag="s_sb")
            TT(s_sb[:], sc_ps,
               maskt[:, bass.ds(i, 1), :, :].squeeze().rearrange("s b t -> s (b t)"),
               ALU.add)
            tap("scores", s_sb[:], l)

            # ---- softmax (bounded scores; wrong-b half masked to exp=0)
            e_sb = ap_.tile([16, B * T], BF16, tag="e_sb")
            S = ap_.tile([16, 1], F32, tag="S")
            ACT(e_sb[:], s_sb[:], ACTF.Exp, accum_out=S[:])
            Sinv = ap_.tile([16, 1], F32, tag="Sinv")
            nc.vector.reciprocal(Sinv[:], S[:])
            p_sb = ap_.tile([16, B * T], BF16, tag="p_sb")
            nc.vector.tensor_scalar_mul(p_sb[:], e_sb[:], Sinv[:])

            # ---- pT: one transpose -> [128 (b t), 16]
            pT_ps = pp.tile([128, 16], BF16, tag="ps")
            nc.tensor.transpose(pT_ps[:], p_sb[:], ident_bf[0:16, 0:16])
            pTs = ap_.tile([128, 16], BF16, tag="pTs")
            CP(pTs[:], pT_ps[:])

            # ---- o matmuls: full-chunk stationary, both hp slots as 2 rhs cols
            o2_ps_t = pp.tile([128, NCK, B, 2], F32, tag="ps")
            o2_ps = o2_ps_t[:]
            for c in range(NCK):
                for b in range(B):
                    s0 = 2 * c + b  # hp=0 slot; hp=1 slot is s0+8
                    nc.tensor.matmul(
                        o2_ps[:, c, b, :],
                        V_row[b * 64:(b + 1) * 64, l, c * 128:(c + 1) * 128],
                        pTs[b * 64:(b + 1) * 64, s0::8][:, 0:2],
                        start=True, stop=True)
            oTs = ap_.tile([128, NCK, B], BF16, tag="oTs")
            CP(oTs[0:64, :, :], o2_ps[0:64, :, :, 0])
            CP(oTs[64:128, :, :], o2_ps[64:128, :, :, 1])
            tap("oTs", oTs[:], l)

            # ---- out projection; z1 = pr + (x_res + b_out) in one TT
            pr_ps = pp.tile([128, NCK, B], F32, tag="ps")
            for mc in range(NCK):
                for kc in range(NCK):
                    nc.tensor.matmul(pr_ps[:, mc, :], w_out[:, l, kc, mc * 128:(mc + 1) * 128],
                                     oTs[:, kc, :], start=(kc == 0), stop=(kc == NCK - 1))
            zs1 = zs_tile("z1")
            z1 = zs1[:, 0]
            TT(z1, pr_ps[:], x_resc[:], ALU.add)

            # ---- LN1 (critical path); z2 = (z1-mu)*A + (bt + ca) directly
            comb1 = stats_pre(zs1, f"s1_{l}")
            a1, am1, t1s = stats_post(comb1, f"s1_{l}", dve=True)
            zs2 = zs_tile("z2")
            z2 = zs2[:, 0]
            materialize(z1, a1, t1s[:, 0:2], l, 0, f"x1m{l}", bias_ap=bca[:], out_ap=z2)
            z2b = ap_.tile([128, NCK, B], BF16, tag="z2b")
            CP(z2b[:], z2)

            # ---- ff1 (folded LN2); stats2/x2 run under it
            comb2 = stats_pre(zs2, f"s2_{l}")
            ff_ps_t = pp.tile([128, NF, B], F32, tag="ps")
            ff_ps = ff_ps_t[:]
            for mc in range(NF):
                for kc in range(NCK):
                    nc.tensor.matmul(ff_ps[:, mc, :], w_ff1[:, l, kc, mc * 128:(mc + 1) * 128],
                                     z2b[:, kc, :], start=(kc == 0), stop=(kc == NCK - 1))
            a2, am2, t2s = stats_post(comb2, f"s2_{l}")
            # x2 + bt2x combined (only used as z3 residual)
            x2c, _ = materialize(z2, a2, t2s[:, 0:2], l, 1, f"x2m{l}",
                                 bias_ap=bt2x[:, l, :, :])
            yb = correct(ff_ps, a2, am2, u_ff1[:, l, :, :], c_ff1[:, l, :, :],
                         NF, f"fc{l}", odt=F32)
            hb = ap_.tile([128, NF, B], BF16, tag="hb")
            nc.vector.tensor_scalar_max(hb[:], yb[:], 0.0)

            # ---- ff2 -> z3 = f2 + (x2 + b_ff2) in one TT
            f2_ps_t = pp.tile([128, NCK, B], F32, tag="ps")
            f2_ps = f2_ps_t[:]
            for mc in range(NCK):
                for kc in range(NF):
                    nc.tensor.matmul(f2_ps[:, mc, :], w_ff2[:, l, kc, mc * 128:(mc + 1) * 128],
                                     hb[:, kc, :], start=(kc == 0), stop=(kc == NF - 1))
            zs3 = zs_tile("z3")
            z3 = zs3[:, 0]
            TT(z3, f2_ps, x2c[:], ALU.add)
            z3b = ap_.tile([128, NCK, B], BF16, tag="z3b")
            CP(z3b[:], z3)
            tap("z3", z3, l)

            comb3 = stats_pre(zs3, f"s3_{l}")
            z_prev, xb = z3, z3b

        # clear vcol column (holds layer-2's v)
        nc.vector.memset(vcol[:, :, :, bass.ds(i, 1)].squeeze(), 0.0)

        # ---- next x0 via fused W_emb (c includes pe[i+1]); stats3 post after
        e_ps_t = pp.tile([128, NCK, B], F32, tag="ps")
        e_ps = e_ps_t[:]
        for mc in range(NCK):
            for kc in range(NCK):
                nc.tensor.matmul(e_ps[:, mc, :], w_emb[:, kc, mc * 128:(mc + 1) * 128],
                                 xb[:, kc, :], start=(kc == 0), stop=(kc == NCK - 1))
        a3, am3, t3 = stats_post(comb3, "s3f")

        # x3[2] materialize -> xfin col i (for deferred mmr)
        x3f, x3fb = materialize(z_prev, a3, t3[:, 0:2], 2, 2, "x3f", out_bf=True)
        CP(xfin[:, :, :, bass.ds(i, 1)].squeeze(), x3fb[:])
        tap("x3", x3f[:])

        t1e = ap_.tile([128, NCK, B], F32, tag="t1e")
        TT(t1e[:], u_emb[:], am3[:].unsqueeze(1).broadcast_to((128, NCK, B)), ALU.mult)
        t2e = ap_.tile([128, NCK, B], F32, tag="t2e")
        TT(t2e[:], c_embpe[:, :, bass.ds(i + 1, 1)].broadcast_to((128, NCK, B)), t1e[:],
           ALU.subtract)
        ya = ap_.tile([128, NCK, B], F32, tag="yae")
        TT(ya[:], e_ps, a3[:].unsqueeze(1).broadcast_to((128, NCK, B)), ALU.mult)
        TT(x0_t[:], ya[:], t2e[:], ALU.add)
        CP(x0b_t[:], x0_t[:])

    if loop_mode == "dyn":
        with tc.For_i(0, n_steps, 1, hint_engines=(mybir.EngineType.PE,)) as i:
            step(i)
    elif loop_mode.startswith("dyn") and loop_mode[3:].isdigit():
        u = int(loop_mode[3:])
        assert n_steps % u == 0
        with tc.For_i(0, n_steps, u,
                      hint_engines=(mybir.EngineType.PE, mybir.EngineType.DVE,
                                    mybir.EngineType.Activation)) as i:
            for j in range(u):
                step(i + j)
    else:  # full static unroll
        for i in range(n_steps):
            step(i)

    # ---- deferred output rows: out = mmr(xfin) batched over all (b, t)
    r_ps = pp.tile([64, B * T], F32, tag="ps")
    for kc in range(NCK):
        nc.tensor.matmul(r_ps[:], w_mmr[:, kc, :],
                         xfin[:, kc, :, :].rearrange("p b t -> p (b t)"),
                         start=(kc == 0), stop=(kc == NCK - 1))
    ro = ap_.tile([64, B * T], F32, tag="ro")
    nc.vector.tensor_scalar_add(ro[:], r_ps[:], b_mmr[:, 0:1])
    fo_ps = pp.tile([128, 64], F32, tag="ps")
    nc.tensor.transpose(fo_ps[:], ro[:], ident_f32[0:64, 0:64])
    fo = ap_.tile([128, 64], F32, tag="fo")
    CP(fo[:], fo_ps[:])
    dma(outs["out"].rearrange("b t m -> (b t) m"), fo[:])

    ctx.close()


# ===================================================================== runner
_CACHE = {}


def _patch_act_tables():
    """Force the table-set chooser to serve Exp AND Ln from the one set that
    contains both (natural_log_exp_and_others) instead of thrashing between
    exp_and_others and natural_log. Only membership info is patched; the
    canonical set ids (dict order) are untouched, so the emitted
    InstLoadActFuncSet ids stay valid."""
    from concourse import bacc as _bacc, hw_specs as _hw
    if getattr(_bacc, "_act_tables_patched", False):
        return
    orig = _hw.get_activation_tables

    def patched(arch):
        tabs = orig(arch)
        keep = "natural_log_exp_and_others"
        if keep in tabs:
            for name, fns in tabs.items():
                if name != keep:
                    fns.discard(ACTF.Exp)
                    fns.discard(ACTF.Ln)
        return tabs

    _bacc.get_activation_tables = patched
    _bacc._act_tables_patched = True


def _build_and_compile(loop_mode="dyn8", n_steps=T):
    key = (loop_mode, n_steps)
    if _CACHE.get("key") == key:
        return
    import concourse.tile as _tile
    from concourse import bacc as _bacc
    _patch_act_tables()
    nc = _bacc.Bacc("TRN2", target_bir_lowering=False, debug=False)
    ins, outs = {}, {}
    for name, (shape, dt) in input_specs().items():
        ins[name] = nc.dram_tensor(name, list(shape), mybir.dt.from_np(np.dtype(dt)),
                                   kind="ExternalInput").ap()
    outs["out"] = nc.dram_tensor("out", [B, T, M], mybir.dt.float32,
                                 kind="ExternalOutput").ap()
    with _tile.TileContext(nc) as tc:
        build(tc, ins, outs, n_steps=n_steps, loop_mode=loop_mode)
    nc.compile()
    _CACHE["nc"] = nc
    _CACHE["key"] = key


def kernel(**inputs):
    """Full (unsharded) inputs -> full output [B, T, M] float32."""
    from concourse.bass_utils import run_bass_kernel_spmd
    _build_and_compile()
    dev_ins = prep_inputs(inputs)
    res = run_bass_kernel_spmd(_CACHE["nc"], [dev_ins], core_ids=[0])
    return np.ascontiguousarray(res.results[0]["out"].astype(np.float32))
